# revision 6
# baseline (speedup 1.0000x reference)
"""Trainium2 Bass kernel for the LIF+STDP spiking net (nn_Net_75084618268982).

Strategy (8 NeuronCores):
  - Shard the hidden dimension H=2048 into 8 shards of 256. Layer-0 dynamics
    (spk0/pre0/post0 over [B=256, I=784]) are replicated on every core; layer-1
    LIF + traces and the STDP weight update of w1 are fully local to each
    core's h-shard (the STDP outer products contract over batch, which every
    core holds in full) -- so the 50-step sequential loop needs NO collectives.
  - Layer 2 (O=10) depends on full H only through cur2 = spk1 @ w2.T; each core
    computes its partial cur2 per step, and ONE AllReduce at the end sums the
    partials; layer-2 LIF runs replicated as a short post-pass.
  - Key algebraic facts used:
      * layer-0 threshold is 0 with subtract-reset => mem0 = 0.9*mem0 + x_t and
        spk0(t) = OR_{tau<=t}(x_tau > 0)  (x >= 0), so mem0 need not be stored.
      * reset(t) = spk(t-1) exactly for the thr=1 "zero" LIF layers.
      * adj_pre <= 0 and adj_post >= 0 always, and 1 + sign(w)*adj > 0, so
        sign(w1) is invariant for the whole run and each clip needs only one
        side: max(.,-1) after update 1 and min(.,1) after update 2.
      * sign fold s*A computed bitwise: B = (w & 0x80000000) ^ A in one fused
        scalar_tensor_tensor op reading A straight from PSUM.
"""

import numpy as np

import concourse.bass as bass
import concourse.mybir as mybir
import concourse.tile as tile
from concourse import bacc
from concourse.bass_utils import run_bass_kernel_spmd
from concourse.masks import make_identity
from concourse.tile_rust import add_dep_helper

F32 = mybir.dt.float32
U32 = mybir.dt.uint32
AOP = mybir.AluOpType
ACT = mybir.ActivationFunctionType

T, B, I, H, O = 50, 256, 784, 2048, 10
NCORES = 8
HS = H // NCORES          # 256
IP = 896                  # I padded to 7*128
KT = IP // 128            # 7 i-tiles
BT = B // 128             # 2 batch tiles
INV_B = 1.0 / B

MM_F32R = False      # use float32r (fast, ~1.2e-4 rel) for the big matmuls
GP_OFF = False       # offload post0/pre1 trace updates to GpSimd


def _build(T=T):
    MDT = mybir.dt.float32r if MM_F32R else F32
    nc = bacc.Bacc("TRN2", target_bir_lowering=False, debug=False,
                   num_devices=NCORES)

    x = nc.dram_tensor("x", (T, B, I), F32, kind="ExternalInput")
    w1ts = nc.dram_tensor("w1ts", (IP, HS), F32, kind="ExternalInput")
    w2ts = nc.dram_tensor("w2ts", (HS, O), F32, kind="ExternalInput")

    o_spk0 = nc.dram_tensor("o_spk0", (T, B, I), F32, kind="ExternalOutput")
    o_pre0 = nc.dram_tensor("o_pre0", (T, B, I), F32, kind="ExternalOutput")
    o_post0 = nc.dram_tensor("o_post0", (T, B, I), F32, kind="ExternalOutput")
    o_spk1 = nc.dram_tensor("o_spk1", (T, B, HS), F32, kind="ExternalOutput")
    o_pre1 = nc.dram_tensor("o_pre1", (T, B, HS), F32, kind="ExternalOutput")
    o_post1 = nc.dram_tensor("o_post1", (T, B, HS), F32, kind="ExternalOutput")
    o_spk2 = nc.dram_tensor("o_spk2", (T, B, O), F32, kind="ExternalOutput")
    o_mem2 = nc.dram_tensor("o_mem2", (T, B, O), F32, kind="ExternalOutput")
    o_pre2 = nc.dram_tensor("o_pre2", (T, B, O), F32, kind="ExternalOutput")
    o_post2 = nc.dram_tensor("o_post2", (T, B, O), F32, kind="ExternalOutput")

    cur2p = nc.dram_tensor("cur2p", (T, 128, BT * O), F32)
    cur2s = nc.dram_tensor("cur2s", (T, 128, BT * O), F32, addr_space="Shared")

    def drt(ap_t, inner):  # [B, F] dram slice -> [128, BT, F] partition view
        return ap_t.rearrange("(bt p) f -> p bt f", p=128)[:, :, :inner]

    with tile.TileContext(nc) as tc:
        with (
            tc.tile_pool(name="pers", bufs=1) as pers,
            tc.tile_pool(name="st0", bufs=2) as st0,
            tc.tile_pool(name="st1", bufs=2) as st1,
            tc.tile_pool(name="scr", bufs=2) as scr,
            tc.tile_pool(name="ps_tp", bufs=2, space="PSUM") as ps_tp,
            tc.tile_pool(name="ps_mm", bufs=1, space="PSUM") as ps_mm,
            tc.tile_pool(name="ps_adj", bufs=4, space="PSUM") as ps_adj,
            tc.tile_pool(name="ps_c2", bufs=1, space="PSUM") as ps_c2,
        ):
            # ---- persistent tiles
            w1T = pers.tile([128, KT, HS], MDT)      # [i_p, i_tile, h] live weights
            w2sb = pers.tile([128, BT, O], MDT)      # [h_p, h_tile, o]
            if MM_F32R:
                wstage = pers.tile([128, KT, HS], F32)
                nc.sync.dma_start(wstage[:],
                                  w1ts.rearrange("(kt p) h -> p kt h", p=128))
                nc.vector.tensor_copy(w1T[:], wstage[:])
                w2stage = pers.tile([128, BT, O], F32)
                nc.sync.dma_start(w2stage[:],
                                  w2ts.rearrange("(kt p) o -> p kt o", p=128))
                nc.vector.tensor_copy(w2sb[:], w2stage[:])
            else:
                nc.sync.dma_start(w1T[:],
                                  w1ts.rearrange("(kt p) h -> p kt h", p=128))
                nc.sync.dma_start(w2sb[:],
                                  w2ts.rearrange("(kt p) o -> p kt o", p=128))
            ident = pers.tile([128, 128], F32)
            make_identity(nc, ident[:])
            smask = pers.tile([128, 1], U32)
            nc.vector.memset(smask[:], 0x80000000)

            # ---- state (double buffered via pools, python vars hold prev AP)
            def zeros(pool, shape, tag, dt_=F32):
                t_ = pool.tile(shape, dt_, tag=tag)
                nc.vector.memset(t_[:].bitcast(F32), 0.0)
                return t_

            spk0 = zeros(st0, [128, BT, IP], "spk0", MDT)
            pre0 = zeros(st0, [128, BT, IP], "pre0", MDT)
            post0 = zeros(st0, [128, BT, IP], "post0")
            mem1 = zeros(st1, [128, BT, HS], "mem1")
            pre1 = zeros(st1, [128, BT, HS], "pre1")
            post1 = zeros(st1, [128, BT, HS], "post1", MDT)
            rmask1 = st1.tile([128, BT, HS], F32, tag="rmask1")
            nc.vector.memset(rmask1[:], 1.0)

            for t in range(T):
                # ---------- input
                xt = scr.tile([128, BT, IP], F32, tag="xt")
                nc.vector.memset(xt[:, :, I:IP], 0.0)
                nc.sync.dma_start(xt[:, :, :I],
                                  x[t].rearrange("(bt p) i -> p bt i", p=128))

                # ---------- layer 0 (replicated)
                spk0_n = st0.tile([128, BT, IP], MDT, tag="spk0")
                nc.vector.scalar_tensor_tensor(
                    spk0_n[:], xt[:], 0.0, spk0[:], op0=AOP.is_gt, op1=AOP.max)
                q0 = scr.tile([128, BT, IP], F32, tag="q0")
                nc.scalar.mul(q0[:], spk0_n[:].bitcast(F32), 0.005)
                pre0_n = st0.tile([128, BT, IP], MDT, tag="pre0")
                nc.vector.scalar_tensor_tensor(
                    pre0_n[:], pre0[:], 0.95, q0[:], op0=AOP.mult, op1=AOP.add)
                post0_n = st0.tile([128, BT, IP], F32, tag="post0")
                if GP_OFF:
                    p0a = scr.tile([128, BT, IP], F32, tag="p0a")
                    nc.gpsimd.tensor_scalar(p0a[:], post0[:], 0.95, None,
                                            op0=AOP.mult)
                    nc.gpsimd.tensor_tensor(post0_n[:], p0a[:], q0[:],
                                            AOP.subtract)
                else:
                    nc.vector.scalar_tensor_tensor(
                        post0_n[:], post0[:], 0.95, q0[:],
                        op0=AOP.mult, op1=AOP.subtract)
                nc.sync.dma_start(drt(o_spk0[t], I), spk0_n[:, :, :I].bitcast(F32))
                nc.sync.dma_start(drt(o_pre0[t], I), pre0_n[:, :, :I].bitcast(F32))
                nc.sync.dma_start(drt(o_post0[t], I), post0_n[:, :, :I])

                # ---------- transpose spk0 -> feature-major [i_p, i_tile, b]
                spk0fm = scr.tile([128, KT, B], MDT, tag="spk0fm")
                for g in range(4):             # groups of up to 4 transposes
                    kk = (2, 2, 2, 1)[g]
                    tp = ps_tp.tile([128, 512], F32, tag="tp")
                    for j in range(2 * kk):
                        k = 2 * g + j // 2
                        bt = j % 2
                        nc.tensor.transpose(
                            tp[:, 128 * j:128 * (j + 1)],
                            spk0_n[:, bt, 128 * k:128 * (k + 1)].bitcast(F32), ident[:])
                    nc.scalar.copy(
                        spk0fm[:, 2 * g:2 * g + kk, :], tp[:, :512 * kk // 2])

                # ---------- matmul1: cur1[b, h] += spk0fm^T @ w1T
                mm1 = ps_mm.tile([128, BT, HS], F32, tag="mm1")
                for bt in range(BT):
                    for k in range(KT):
                        nc.tensor.matmul(
                            mm1[:, bt, :],
                            spk0fm[:, k, 128 * bt:128 * (bt + 1)],
                            w1T[:, k, :],
                            start=(k == 0), stop=(k == KT - 1))
                cur1 = scr.tile([128, BT, HS], F32, tag="cur1")
                nc.scalar.activation(cur1[:], mm1[:], ACT.Relu)

                # ---------- layer 1 LIF (zero-reset, thr=1)
                m1a = scr.tile([128, BT, HS], F32, tag="m1a")
                nc.vector.scalar_tensor_tensor(
                    m1a[:], mem1[:], 0.9, cur1[:], op0=AOP.mult, op1=AOP.add)
                mem1_n = st1.tile([128, BT, HS], F32, tag="mem1")
                nc.vector.tensor_tensor(mem1_n[:], m1a[:], rmask1[:], AOP.mult)
                spk1_n = st1.tile([128, BT, HS], MDT, tag="spk1")
                nc.vector.tensor_scalar(spk1_n[:], mem1_n[:], 1.0, None,
                                        op0=AOP.is_gt)
                q1 = scr.tile([128, BT, HS], F32, tag="q1")
                nc.vector.tensor_scalar(q1[:], mem1_n[:], 1.0, 0.005,
                                        op0=AOP.is_gt, op1=AOP.mult)
                rmask1_n = st1.tile([128, BT, HS], F32, tag="rmask1")
                nc.vector.tensor_scalar(rmask1_n[:], mem1_n[:], 1.0, None,
                                        op0=AOP.is_le)
                pre1_n = st1.tile([128, BT, HS], F32, tag="pre1")
                if GP_OFF:
                    p1a = scr.tile([128, BT, HS], F32, tag="p1a")
                    nc.gpsimd.tensor_scalar(p1a[:], pre1[:], 0.95, None,
                                            op0=AOP.mult)
                    nc.gpsimd.tensor_tensor(pre1_n[:], p1a[:], q1[:], AOP.add)
                else:
                    nc.vector.scalar_tensor_tensor(
                        pre1_n[:], pre1[:], 0.95, q1[:],
                        op0=AOP.mult, op1=AOP.add)
                post1_n = st1.tile([128, BT, HS], MDT, tag="post1")
                nc.vector.scalar_tensor_tensor(
                    post1_n[:], post1[:], 0.95, q1[:],
                    op0=AOP.mult, op1=AOP.subtract)
                nc.sync.dma_start(drt(o_spk1[t], HS), spk1_n[:].bitcast(F32))
                nc.sync.dma_start(drt(o_pre1[t], HS), pre1_n[:])
                nc.sync.dma_start(drt(o_post1[t], HS), post1_n[:].bitcast(F32))

                # ---------- spk1 -> feature-major, cur2 partial
                spk1fm = scr.tile([128, BT, B], MDT, tag="spk1fm")
                tp1 = ps_tp.tile([128, 512], F32, tag="tp")
                for j in range(2):
                    for bt in range(BT):
                        nc.tensor.transpose(
                            tp1[:, 128 * (2 * j + bt):128 * (2 * j + bt + 1)],
                            spk1_n[:, bt, 128 * j:128 * (j + 1)].bitcast(F32), ident[:])
                nc.scalar.copy(spk1fm[:], tp1[:])
                c2 = ps_c2.tile([128, BT, O], F32, tag="c2")
                for bt in range(BT):
                    for k in range(BT):
                        nc.tensor.matmul(
                            c2[:, bt, :],
                            spk1fm[:, k, 128 * bt:128 * (bt + 1)],
                            w2sb[:, k, :],
                            start=(k == 0), stop=(k == BT - 1))
                c2sb = scr.tile([128, BT, O], F32, tag="c2sb")
                nc.scalar.copy(c2sb[:], c2[:])
                nc.sync.dma_start(
                    cur2p[t].rearrange("p (bt o) -> p bt o", bt=BT), c2sb[:])

                # ---------- STDP update 1: w += |w| * (post1^T spk0)/B ; max(.,-1)
                def stdp_update(lhs_bm, rhs_bm, final_op, final_const):
                    # A[i, h] = sum_b lhs_bm[b, i] * rhs_bm[b, h] into PSUM
                    chunks = []
                    for c in range(4):
                        kk = (2, 2, 2, 1)[c]
                        pa = ps_adj.tile([128, 2, HS], F32, tag="adj")
                        for s in range(kk):
                            m = 2 * c + s
                            for bt in range(BT):
                                nc.tensor.matmul(
                                    pa[:, s, :],
                                    lhs_bm[:, bt, 128 * m:128 * (m + 1)],
                                    rhs_bm[:, bt, :],
                                    start=(bt == 0), stop=(bt == BT - 1))
                        chunks.append((pa, kk))
                    bw = scr.tile([128, KT, HS], F32, tag="bw")
                    for c, (pa, kk) in enumerate(chunks):
                        nc.vector.scalar_tensor_tensor(
                            bw[:, 2 * c:2 * c + kk, :].bitcast(U32),
                            w1T[:, 2 * c:2 * c + kk, :].bitcast(U32),
                            smask[:], pa[:, :kk, :].bitcast(U32),
                            op0=AOP.bitwise_and, op1=AOP.bitwise_xor)
                    nc.vector.tensor_scalar(bw[:], bw[:], INV_B, 1.0,
                                            op0=AOP.mult, op1=AOP.add)
                    nc.vector.tensor_tensor(w1T[:], bw[:], w1T[:], AOP.mult)
                    if final_op is AOP.max:
                        nc.vector.tensor_scalar_max(w1T[:], w1T[:], final_const)
                    else:
                        nc.vector.tensor_scalar_min(w1T[:], w1T[:], final_const)

                stdp_update(spk0_n, post1_n, AOP.max, -1.0)
                stdp_update(pre0_n, spk1_n, AOP.min, 1.0)

                spk0, pre0, post0 = spk0_n, pre0_n, post0_n
                mem1, pre1, post1, rmask1 = mem1_n, pre1_n, post1_n, rmask1_n

            # ================= phase 2: layer 2 =================
            cc = nc.gpsimd.collective_compute(
                "AllReduce", AOP.add,
                replica_groups=[list(range(NCORES))],
                ins=[cur2p[:].opt()], outs=[cur2s[:].opt()])

            c2all = pers.tile([128, T, BT * O], F32)
            din = nc.sync.dma_start(c2all[:], cur2s.rearrange("t p c -> p t c"))
            add_dep_helper(din.ins, cc.ins, reason="load AllReduce result")
            nc.vector.tensor_scalar_max(c2all[:], c2all[:], 0.0)  # relu

            mem2 = zeros(st1, [128, BT, O], "mem2")
            pre2 = zeros(st1, [128, BT, O], "pre2")
            post2 = zeros(st1, [128, BT, O], "post2")
            rmask2 = st1.tile([128, BT, O], F32, tag="rmask2")
            nc.vector.memset(rmask2[:], 1.0)

            for t in range(T):
                rt = c2all[:, t, :].rearrange("p (bt o) -> p bt o", bt=BT)
                m2a = scr.tile([128, BT, O], F32, tag="m2a")
                nc.vector.scalar_tensor_tensor(
                    m2a[:], mem2[:], 0.9, rt, op0=AOP.mult, op1=AOP.add)
                mem2_n = st1.tile([128, BT, O], F32, tag="mem2")
                nc.vector.tensor_tensor(mem2_n[:], m2a[:], rmask2[:], AOP.mult)
                spk2_n = st1.tile([128, BT, O], F32, tag="spk2")
                nc.vector.tensor_scalar(spk2_n[:], mem2_n[:], 1.0, None,
                                        op0=AOP.is_gt)
                q2 = scr.tile([128, BT, O], F32, tag="q2")
                nc.vector.tensor_scalar(q2[:], mem2_n[:], 1.0, 0.005,
                                        op0=AOP.is_gt, op1=AOP.mult)
                rmask2_n = st1.tile([128, BT, O], F32, tag="rmask2")
                nc.vector.tensor_scalar(rmask2_n[:], mem2_n[:], 1.0, None,
                                        op0=AOP.is_le)
                pre2_n = st1.tile([128, BT, O], F32, tag="pre2")
                nc.vector.scalar_tensor_tensor(
                    pre2_n[:], pre2[:], 0.95, q2[:], op0=AOP.mult, op1=AOP.add)
                post2_n = st1.tile([128, BT, O], F32, tag="post2")
                nc.vector.scalar_tensor_tensor(
                    post2_n[:], post2[:], 0.95, q2[:],
                    op0=AOP.mult, op1=AOP.subtract)
                nc.sync.dma_start(drt(o_spk2[t], O), spk2_n[:])
                nc.sync.dma_start(drt(o_mem2[t], O), mem2_n[:])
                nc.sync.dma_start(drt(o_pre2[t], O), pre2_n[:])
                nc.sync.dma_start(drt(o_post2[t], O), post2_n[:])
                mem2, pre2, post2, rmask2 = mem2_n, pre2_n, post2_n, rmask2_n

    nc.finalize()
    return nc


_NC_CACHE = None


def kernel(x, w1, w2):
    global _NC_CACHE
    if _NC_CACHE is None:
        _NC_CACHE = _build()
    nc = _NC_CACHE

    x = np.ascontiguousarray(x, dtype=np.float32)
    in_maps = []
    for c in range(NCORES):
        w1s = np.zeros((IP, HS), dtype=np.float32)
        w1s[:I, :] = w1[c * HS:(c + 1) * HS, :].T
        w2s = np.ascontiguousarray(w2[:, c * HS:(c + 1) * HS].T,
                                   dtype=np.float32)
        in_maps.append({"x": x, "w1ts": w1s, "w2ts": w2s})

    res = run_bass_kernel_spmd(nc, in_maps, core_ids=list(range(NCORES)))
    r0 = res.results[0]

    spk1 = np.concatenate([res.results[c]["o_spk1"] for c in range(NCORES)],
                          axis=2)
    pre1 = np.concatenate([res.results[c]["o_pre1"] for c in range(NCORES)],
                          axis=2)
    post1 = np.concatenate([res.results[c]["o_post1"] for c in range(NCORES)],
                           axis=2)

    return (r0["o_spk0"], spk1, r0["o_spk2"], r0["o_mem2"],
            r0["o_pre0"], pre1, r0["o_pre2"],
            r0["o_post0"], post1, r0["o_post2"])


# revision 15
# speedup vs baseline: 1.0127x; 1.0127x over previous
"""Trainium2 Bass kernel for the LIF+STDP spiking net (nn_Net_75084618268982).

Strategy (8 NeuronCores):
  - Shard the hidden dimension H=2048 into 8 shards of 256. Layer-0 dynamics
    (spk0/pre0/post0 over [B=256, I=784]) are replicated on every core; layer-1
    LIF + traces and the STDP weight update of w1 are fully local to each
    core's h-shard (the STDP outer products contract over batch, which every
    core holds in full) -- so the 50-step sequential loop needs NO collectives.
  - Layer 2 (O=10) depends on full H only through cur2 = spk1 @ w2.T; each core
    computes its partial cur2 per step, and ONE AllReduce at the end sums the
    partials; layer-2 LIF runs replicated as a short post-pass.
  - Key algebraic facts used:
      * layer-0 threshold is 0 with subtract-reset => mem0 = 0.9*mem0 + x_t and
        spk0(t) = OR_{tau<=t}(x_tau > 0)  (x >= 0), so mem0 need not be stored.
      * reset(t) = spk(t-1) exactly for the thr=1 "zero" LIF layers.
      * adj_pre <= 0 and adj_post >= 0 always, and 1 + sign(w)*adj > 0, so
        sign(w1) is invariant for the whole run and each clip needs only one
        side: max(.,-1) after update 1 and min(.,1) after update 2.
      * sign fold s*A computed bitwise: B = (w & 0x80000000) ^ A in one fused
        scalar_tensor_tensor op reading A straight from PSUM.
"""

import numpy as np

import concourse.bass as bass
import concourse.mybir as mybir
import concourse.tile as tile
from concourse import bacc
from concourse.bass_utils import run_bass_kernel_spmd
from concourse.masks import make_identity
from concourse.tile_rust import add_dep_helper

F32 = mybir.dt.float32
U32 = mybir.dt.uint32
AOP = mybir.AluOpType
ACT = mybir.ActivationFunctionType

T, B, I, H, O = 50, 256, 784, 2048, 10
NCORES = 8
HS = H // NCORES          # 256
IP = 896                  # I padded to 7*128
KT = IP // 128            # 7 i-tiles
BT = B // 128             # 2 batch tiles
INV_B = 1.0 / B

import os
MM_F32R = os.environ.get("K_MM_F32R", "0") == "1"   # float32r big matmuls
GP_OFF = os.environ.get("K_GP_OFF", "0") == "1"     # GpSimd offload
PE_WARM = os.environ.get("K_PE_WARM", "0") == "1"   # dummy matmuls to keep HAM hot


def _build(T=T):
    MDT = mybir.dt.float32r if MM_F32R else F32
    nc = bacc.Bacc("TRN2", target_bir_lowering=False, debug=False,
                   num_devices=NCORES)

    x = nc.dram_tensor("x", (T, B, I), F32, kind="ExternalInput")
    w1ts = nc.dram_tensor("w1ts", (IP, HS), F32, kind="ExternalInput")
    w2ts = nc.dram_tensor("w2ts", (HS, O), F32, kind="ExternalInput")

    o_spk0 = nc.dram_tensor("o_spk0", (T, B, I), F32, kind="ExternalOutput")
    o_pre0 = nc.dram_tensor("o_pre0", (T, B, I), F32, kind="ExternalOutput")
    o_post0 = nc.dram_tensor("o_post0", (T, B, I), F32, kind="ExternalOutput")
    o_spk1 = nc.dram_tensor("o_spk1", (T, B, HS), F32, kind="ExternalOutput")
    o_pre1 = nc.dram_tensor("o_pre1", (T, B, HS), F32, kind="ExternalOutput")
    o_post1 = nc.dram_tensor("o_post1", (T, B, HS), F32, kind="ExternalOutput")
    o_spk2 = nc.dram_tensor("o_spk2", (T, B, O), F32, kind="ExternalOutput")
    o_mem2 = nc.dram_tensor("o_mem2", (T, B, O), F32, kind="ExternalOutput")
    o_pre2 = nc.dram_tensor("o_pre2", (T, B, O), F32, kind="ExternalOutput")
    o_post2 = nc.dram_tensor("o_post2", (T, B, O), F32, kind="ExternalOutput")

    cur2p = nc.dram_tensor("cur2p", (T, 128, BT * O), F32)
    cur2s = nc.dram_tensor("cur2s", (T, 128, BT * O), F32, addr_space="Shared")

    def drt(ap_t, inner):  # [B, F] dram slice -> [128, BT, F] partition view
        return ap_t.rearrange("(bt p) f -> p bt f", p=128)[:, :, :inner]

    with tile.TileContext(nc) as tc:
        with (
            tc.tile_pool(name="pers", bufs=1) as pers,
            tc.tile_pool(name="st0", bufs=3) as st0,
            tc.tile_pool(name="scr3", bufs=3) as scr3,
            tc.tile_pool(name="st1", bufs=2) as st1,
            tc.tile_pool(name="scr", bufs=2) as scr,
            tc.tile_pool(name="ps_tp", bufs=2, space="PSUM") as ps_tp,
            tc.tile_pool(name="ps_mm", bufs=1, space="PSUM") as ps_mm,
            tc.tile_pool(name="ps_adj", bufs=4, space="PSUM") as ps_adj,
            tc.tile_pool(name="ps_c2", bufs=1, space="PSUM") as ps_c2,
        ):
            warm_ps = None
            # ---- persistent tiles
            w1T = pers.tile([128, KT, HS], MDT)      # [i_p, i_tile, h] live weights
            w2sb = pers.tile([128, BT, O], MDT)      # [h_p, h_tile, o]
            if MM_F32R:
                wstage = pers.tile([128, KT, HS], F32)
                nc.sync.dma_start(wstage[:],
                                  w1ts.rearrange("(kt p) h -> p kt h", p=128))
                nc.vector.tensor_copy(w1T[:], wstage[:])
                w2stage = pers.tile([128, BT, O], F32)
                nc.sync.dma_start(w2stage[:],
                                  w2ts.rearrange("(kt p) o -> p kt o", p=128))
                nc.vector.tensor_copy(w2sb[:], w2stage[:])
            else:
                nc.sync.dma_start(w1T[:],
                                  w1ts.rearrange("(kt p) h -> p kt h", p=128))
                nc.sync.dma_start(w2sb[:],
                                  w2ts.rearrange("(kt p) o -> p kt o", p=128))
            ident = pers.tile([128, 128], F32)
            make_identity(nc, ident[:])
            smask = pers.tile([128, 1], U32)
            nc.vector.memset(smask[:], 0x80000000)

            # ---- state (double buffered via pools, python vars hold prev AP)
            def zeros(pool, shape, tag, dt_=F32):
                t_ = pool.tile(shape, dt_, tag=tag)
                nc.vector.memset(t_[:].bitcast(F32), 0.0)
                return t_

            spk0 = zeros(st0, [128, BT, IP], "spk0", MDT)
            pre0 = zeros(st0, [128, BT, IP], "pre0", MDT)
            post0 = zeros(st0, [128, BT, IP], "post0")
            mem1 = zeros(st1, [128, BT, HS], "mem1")
            pre1 = zeros(st1, [128, BT, HS], "pre1")
            post1 = zeros(st1, [128, BT, HS], "post1", MDT)
            rmask1 = st1.tile([128, BT, HS], F32, tag="rmask1")
            nc.vector.memset(rmask1[:], 1.0)

            def emit_l0(t, spk0, pre0, post0):
                """Layer-0 for step t (pure function of x): returns new tiles."""
                xt = scr3.tile([128, BT, IP], F32, tag="xt")
                nc.sync.dma_start(xt[:, :, :I],
                                  x[t].rearrange("(bt p) i -> p bt i", p=128))
                spk0_n = st0.tile([128, BT, IP], MDT, tag="spk0")
                nc.vector.scalar_tensor_tensor(
                    spk0_n[:], xt[:], 0.0, spk0[:], op0=AOP.is_gt, op1=AOP.max)
                q0 = scr3.tile([128, BT, IP], F32, tag="q0")
                nc.scalar.mul(q0[:], spk0_n[:].bitcast(F32), 0.005)
                pre0_n = st0.tile([128, BT, IP], MDT, tag="pre0")
                nc.vector.scalar_tensor_tensor(
                    pre0_n[:], pre0[:], 0.95, q0[:], op0=AOP.mult, op1=AOP.add)
                post0_n = st0.tile([128, BT, IP], F32, tag="post0")
                if GP_OFF:
                    p0a = scr.tile([128, BT, IP], F32, tag="p0a")
                    nc.gpsimd.tensor_scalar(p0a[:], post0[:], 0.95, None,
                                            op0=AOP.mult)
                    nc.gpsimd.tensor_tensor(post0_n[:], p0a[:], q0[:],
                                            AOP.subtract)
                else:
                    nc.vector.scalar_tensor_tensor(
                        post0_n[:], post0[:], 0.95, q0[:],
                        op0=AOP.mult, op1=AOP.subtract)
                nc.sync.dma_start(drt(o_spk0[t], I),
                                  spk0_n[:, :, :I].bitcast(F32))
                nc.sync.dma_start(drt(o_pre0[t], I),
                                  pre0_n[:, :, :I].bitcast(F32))
                nc.sync.dma_start(drt(o_post0[t], I), post0_n[:, :, :I])
                return spk0_n, pre0_n, post0_n

            # pre-zero the pad columns of all xt slots once (outside the
            # loop so the per-step DMA isn't WAW-chained to a DVE memset)
            for _ in range(3):
                xt_slot = scr3.tile([128, BT, IP], F32, tag="xt")
                nc.vector.memset(xt_slot[:, :, I:IP], 0.0)

            # prologue: layer-0 for step 0
            spk0_n, pre0_n, post0_n = emit_l0(0, spk0, pre0, post0)

            for t in range(T):
                # ---------- software-pipelined layer 0 for step t+1 (emitted
                # first so it can fill DVE bubbles during matmul1)
                spk0_c, pre0_c = spk0_n, pre0_n
                if t + 1 < T:
                    spk0_n, pre0_n, post0_n = emit_l0(
                        t + 1, spk0_n, pre0_n, post0_n)

                # ---------- transpose spk0 -> feature-major [i_p, i_tile, b]
                spk0fm = scr.tile([128, KT, B], MDT, tag="spk0fm")
                for g in range(4):             # groups of up to 4 transposes
                    kk = (2, 2, 2, 1)[g]
                    tp = ps_tp.tile([128, 512], F32, tag="tp")
                    for j in range(2 * kk):
                        k = 2 * g + j // 2
                        bt = j % 2
                        nc.tensor.transpose(
                            tp[:, 128 * j:128 * (j + 1)],
                            spk0_c[:, bt, 128 * k:128 * (k + 1)].bitcast(F32), ident[:])
                    nc.scalar.copy(
                        spk0fm[:, 2 * g:2 * g + kk, :], tp[:, :512 * kk // 2])

                # ---------- matmul1: cur1[b, h] += spk0fm^T @ w1T
                mm1 = ps_mm.tile([128, BT, HS], F32, tag="mm1")
                for bt in range(BT):
                    for k in range(KT):
                        nc.tensor.matmul(
                            mm1[:, bt, :],
                            spk0fm[:, k, 128 * bt:128 * (bt + 1)],
                            w1T[:, k, :],
                            start=(k == 0), stop=(k == KT - 1))
                cur1 = scr.tile([128, BT, HS], F32, tag="cur1")
                nc.scalar.activation(cur1[:], mm1[:], ACT.Relu)

                # ---------- layer 1 LIF (zero-reset, thr=1)
                m1a = scr.tile([128, BT, HS], F32, tag="m1a")
                nc.vector.scalar_tensor_tensor(
                    m1a[:], mem1[:], 0.9, cur1[:], op0=AOP.mult, op1=AOP.add)
                mem1_n = st1.tile([128, BT, HS], F32, tag="mem1")
                nc.vector.tensor_tensor(mem1_n[:], m1a[:], rmask1[:], AOP.mult)
                spk1_n = st1.tile([128, BT, HS], MDT, tag="spk1")
                nc.vector.tensor_scalar(spk1_n[:], mem1_n[:], 1.0, None,
                                        op0=AOP.is_gt)
                q1 = scr.tile([128, BT, HS], F32, tag="q1")
                nc.vector.tensor_scalar(q1[:], mem1_n[:], 1.0, 0.005,
                                        op0=AOP.is_gt, op1=AOP.mult)
                rmask1_n = st1.tile([128, BT, HS], F32, tag="rmask1")
                nc.vector.tensor_scalar(rmask1_n[:], mem1_n[:], 1.0, None,
                                        op0=AOP.is_le)
                pre1_n = st1.tile([128, BT, HS], F32, tag="pre1")
                if GP_OFF:
                    p1a = scr.tile([128, BT, HS], F32, tag="p1a")
                    nc.gpsimd.tensor_scalar(p1a[:], pre1[:], 0.95, None,
                                            op0=AOP.mult)
                    nc.gpsimd.tensor_tensor(pre1_n[:], p1a[:], q1[:], AOP.add)
                else:
                    nc.vector.scalar_tensor_tensor(
                        pre1_n[:], pre1[:], 0.95, q1[:],
                        op0=AOP.mult, op1=AOP.add)
                post1_n = st1.tile([128, BT, HS], MDT, tag="post1")
                nc.vector.scalar_tensor_tensor(
                    post1_n[:], post1[:], 0.95, q1[:],
                    op0=AOP.mult, op1=AOP.subtract)
                nc.sync.dma_start(drt(o_spk1[t], HS), spk1_n[:].bitcast(F32))
                nc.sync.dma_start(drt(o_pre1[t], HS), pre1_n[:])
                nc.sync.dma_start(drt(o_post1[t], HS), post1_n[:].bitcast(F32))

                # ---------- spk1 -> feature-major, cur2 partial
                spk1fm = scr.tile([128, BT, B], MDT, tag="spk1fm")
                tp1 = ps_tp.tile([128, 512], F32, tag="tp")
                for j in range(2):
                    for bt in range(BT):
                        nc.tensor.transpose(
                            tp1[:, 128 * (2 * j + bt):128 * (2 * j + bt + 1)],
                            spk1_n[:, bt, 128 * j:128 * (j + 1)].bitcast(F32), ident[:])
                nc.scalar.copy(spk1fm[:], tp1[:])
                c2 = ps_c2.tile([128, BT, O], F32, tag="c2")
                for bt in range(BT):
                    for k in range(BT):
                        nc.tensor.matmul(
                            c2[:, bt, :],
                            spk1fm[:, k, 128 * bt:128 * (bt + 1)],
                            w2sb[:, k, :],
                            start=(k == 0), stop=(k == BT - 1))
                c2sb = scr.tile([128, BT, O], F32, tag="c2sb")
                nc.scalar.copy(c2sb[:], c2[:])
                nc.sync.dma_start(
                    cur2p[t].rearrange("p (bt o) -> p bt o", bt=BT), c2sb[:])

                # ---------- STDP update 1: w += |w| * (post1^T spk0)/B ; max(.,-1)
                def stdp_update(lhs_bm, rhs_bm, final_op, final_const):
                    # A[i, h] = sum_b lhs_bm[b, i] * rhs_bm[b, h] into PSUM
                    chunks = []
                    for c in range(4):
                        kk = (2, 2, 2, 1)[c]
                        pa = ps_adj.tile([128, 2, HS], F32, tag="adj")
                        for s in range(kk):
                            m = 2 * c + s
                            for bt in range(BT):
                                nc.tensor.matmul(
                                    pa[:, s, :],
                                    lhs_bm[:, bt, 128 * m:128 * (m + 1)],
                                    rhs_bm[:, bt, :],
                                    start=(bt == 0), stop=(bt == BT - 1))
                        chunks.append((pa, kk))
                    bw = scr.tile([128, KT, HS], F32, tag="bw")
                    for c, (pa, kk) in enumerate(chunks):
                        nc.vector.scalar_tensor_tensor(
                            bw[:, 2 * c:2 * c + kk, :].bitcast(U32),
                            w1T[:, 2 * c:2 * c + kk, :].bitcast(U32),
                            smask[:], pa[:, :kk, :].bitcast(U32),
                            op0=AOP.bitwise_and, op1=AOP.bitwise_xor)
                    nc.vector.tensor_scalar(bw[:], bw[:], INV_B, 1.0,
                                            op0=AOP.mult, op1=AOP.add)
                    nc.vector.tensor_tensor(w1T[:], bw[:], w1T[:], AOP.mult)
                    if final_op is AOP.max:
                        nc.vector.tensor_scalar_max(w1T[:], w1T[:], final_const)
                    else:
                        nc.vector.tensor_scalar_min(w1T[:], w1T[:], final_const)

                stdp_update(spk0_c, post1_n, AOP.max, -1.0)
                stdp_update(pre0_c, spk1_n, AOP.min, 1.0)

                if PE_WARM:
                    # low-priority junk matmuls: keep the PE HAM un-throttled
                    # through the DVE-heavy stretch of each step
                    warm_ps = ps_tp.tile([128, 512], F32, tag="tp")
                    for _ in range(8):
                        nc.tensor.matmul(warm_ps[:, :64], ident[:],
                                         ident[:, :64],
                                         start=True, stop=True)

                mem1, pre1, post1, rmask1 = mem1_n, pre1_n, post1_n, rmask1_n

            # ================= phase 2: layer 2 =================
            cc = nc.gpsimd.collective_compute(
                "AllReduce", AOP.add,
                replica_groups=[list(range(NCORES))],
                ins=[cur2p[:].opt()], outs=[cur2s[:].opt()])

            c2all = pers.tile([128, T, BT * O], F32)
            din = nc.sync.dma_start(c2all[:], cur2s.rearrange("t p c -> p t c"))
            add_dep_helper(din.ins, cc.ins, reason="load AllReduce result")
            nc.vector.tensor_scalar_max(c2all[:], c2all[:], 0.0)  # relu

            mem2 = zeros(st1, [128, BT, O], "mem2")
            pre2 = zeros(st1, [128, BT, O], "pre2")
            post2 = zeros(st1, [128, BT, O], "post2")
            rmask2 = st1.tile([128, BT, O], F32, tag="rmask2")
            nc.vector.memset(rmask2[:], 1.0)

            for t in range(T):
                rt = c2all[:, t, :].rearrange("p (bt o) -> p bt o", bt=BT)
                m2a = scr.tile([128, BT, O], F32, tag="m2a")
                nc.vector.scalar_tensor_tensor(
                    m2a[:], mem2[:], 0.9, rt, op0=AOP.mult, op1=AOP.add)
                mem2_n = st1.tile([128, BT, O], F32, tag="mem2")
                nc.vector.tensor_tensor(mem2_n[:], m2a[:], rmask2[:], AOP.mult)
                spk2_n = st1.tile([128, BT, O], F32, tag="spk2")
                nc.vector.tensor_scalar(spk2_n[:], mem2_n[:], 1.0, None,
                                        op0=AOP.is_gt)
                q2 = scr.tile([128, BT, O], F32, tag="q2")
                nc.vector.tensor_scalar(q2[:], mem2_n[:], 1.0, 0.005,
                                        op0=AOP.is_gt, op1=AOP.mult)
                rmask2_n = st1.tile([128, BT, O], F32, tag="rmask2")
                nc.vector.tensor_scalar(rmask2_n[:], mem2_n[:], 1.0, None,
                                        op0=AOP.is_le)
                pre2_n = st1.tile([128, BT, O], F32, tag="pre2")
                nc.vector.scalar_tensor_tensor(
                    pre2_n[:], pre2[:], 0.95, q2[:], op0=AOP.mult, op1=AOP.add)
                post2_n = st1.tile([128, BT, O], F32, tag="post2")
                nc.vector.scalar_tensor_tensor(
                    post2_n[:], post2[:], 0.95, q2[:],
                    op0=AOP.mult, op1=AOP.subtract)
                nc.sync.dma_start(drt(o_spk2[t], O), spk2_n[:])
                nc.sync.dma_start(drt(o_mem2[t], O), mem2_n[:])
                nc.sync.dma_start(drt(o_pre2[t], O), pre2_n[:])
                nc.sync.dma_start(drt(o_post2[t], O), post2_n[:])
                mem2, pre2, post2, rmask2 = mem2_n, pre2_n, post2_n, rmask2_n

    nc.finalize()
    return nc


_NC_CACHE = None


def kernel(x, w1, w2):
    global _NC_CACHE
    if _NC_CACHE is None:
        _NC_CACHE = _build()
    nc = _NC_CACHE

    x = np.ascontiguousarray(x, dtype=np.float32)
    in_maps = []
    for c in range(NCORES):
        w1s = np.zeros((IP, HS), dtype=np.float32)
        w1s[:I, :] = w1[c * HS:(c + 1) * HS, :].T
        w2s = np.ascontiguousarray(w2[:, c * HS:(c + 1) * HS].T,
                                   dtype=np.float32)
        in_maps.append({"x": x, "w1ts": w1s, "w2ts": w2s})

    res = run_bass_kernel_spmd(nc, in_maps, core_ids=list(range(NCORES)))
    r0 = res.results[0]

    spk1 = np.concatenate([res.results[c]["o_spk1"] for c in range(NCORES)],
                          axis=2)
    pre1 = np.concatenate([res.results[c]["o_pre1"] for c in range(NCORES)],
                          axis=2)
    post1 = np.concatenate([res.results[c]["o_post1"] for c in range(NCORES)],
                           axis=2)

    return (r0["o_spk0"], spk1, r0["o_spk2"], r0["o_mem2"],
            r0["o_pre0"], pre1, r0["o_pre2"],
            r0["o_post0"], post1, r0["o_post2"])
